# revision 1
# baseline (speedup 1.0000x reference)
"""Trainium2 Bass kernel for the autoregressive VAE (3-layer enc/dec LSTM).

Strategy: 8-way tensor parallelism over the hidden/gate dimension.
Core k owns h-indices [128k:128k+128) of every LSTM layer (enc+dec).
Per layer+step it computes its [B=64, 512] gate slice (batch-major
matmuls: activations stationary, weights moving, 2x column tiling),
runs the cell elementwise update for its h-slice, transposes h to
[128, 64] and all-gathers h.T across the 8 cores (needed both by the
next layer's input matmul and this layer's next-step recurrence).
Heads (mu/logvar/z/logits) are replicated on every core so the
z -> decoder and logits -> next-step-encoder feedbacks need no
communication.  Weights live in SBUF as bf16 for the whole kernel
(~12MB/core); cell state c stays fp32.
"""

import sys

sys.path.insert(0, "/opt/trn_rl_repo")

import numpy as np
import ml_dtypes

from concourse import bass, tile, mybir, bacc
from concourse.bass_utils import run_bass_kernel_spmd

BF16 = ml_dtypes.bfloat16
L, H, D, B, T_FULL = 3, 1024, 256, 64, 128
NC = 8
SL = H // NC          # 128 h-indices per core
G = 4 * SL            # 512 gate rows per core
AF = mybir.ActivationFunctionType


def _chunked_moving(WT, n_cols):
    """[K, n_cols] -> SBUF layout [128, (K//128)*n_cols] bf16, chunk-major."""
    K = WT.shape[0]
    assert K % 128 == 0
    return (
        WT.reshape(K // 128, 128, n_cols)
        .transpose(1, 0, 2)
        .reshape(128, (K // 128) * n_cols)
        .astype(BF16)
    )


def prepare_core_inputs(inputs, core, T=T_FULL):
    """Host-side preprocessing of one core's input map."""
    i = inputs
    rows = np.concatenate(
        [np.arange(g * H + SL * core, g * H + SL * core + SL) for g in range(4)]
    )
    m = {}
    layers = [
        ("e0", i["enc_Wih0"], i["enc_Whh"][0], i["enc_b"][0]),
        ("e1", i["enc_Wih"][0], i["enc_Whh"][1], i["enc_b"][1]),
        ("e2", i["enc_Wih"][1], i["enc_Whh"][2], i["enc_b"][2]),
        ("d0", i["dec_Wih0"], i["dec_Whh"][0], i["dec_b"][0]),
        ("d1", i["dec_Wih"][0], i["dec_Whh"][1], i["dec_b"][1]),
        ("d2", i["dec_Wih"][1], i["dec_Whh"][2], i["dec_b"][2]),
    ]
    for name, Wih, Whh, b in layers:
        m[f"wih_{name}"] = _chunked_moving(np.ascontiguousarray(Wih[rows].T), G)
        m[f"whh_{name}"] = _chunked_moving(np.ascontiguousarray(Whh[rows].T), G)
        m[f"b_{name}"] = b[rows].astype(BF16).reshape(1, G)
    # heads, replicated on every core
    m["w_mu"] = _chunked_moving(np.ascontiguousarray(i["W_mu"].T), D)
    m["w_lv"] = _chunked_moving(np.ascontiguousarray(i["W_logvar"].T), D)
    m["w_out"] = _chunked_moving(np.ascontiguousarray(i["W_out"].T), D)
    m["b_mu"] = i["b_mu"].astype(BF16).reshape(1, D)
    m["b_lv"] = i["b_logvar"].astype(BF16).reshape(1, D)
    m["b_out"] = i["b_out"].astype(BF16).reshape(1, D)
    # x transposed for stationary use: [T, 128, 2, 64] (partition-major)
    xT = (
        i["x"][:, :T]
        .transpose(1, 2, 0)               # [T, D, B]
        .reshape(T, 2, 128, B)
        .transpose(0, 2, 1, 3)            # [T, 128, 2, B]
        .astype(BF16)
        .reshape(T, 128, 2 * B)
    )
    m["xT"] = np.ascontiguousarray(xT)
    m["x_f"] = np.ascontiguousarray(i["x"][:, :T].transpose(1, 0, 2)).astype(np.float32)
    m["eps_f"] = np.ascontiguousarray(i["eps"][:, :T].transpose(1, 0, 2)).astype(
        np.float32
    )
    m["ones"] = np.ones((1, B), BF16)
    m["ident"] = np.eye(128, dtype=BF16)
    return m


def build(T=T_FULL, steps=None, gather_mode='cc', cell_mode='full'):
    nc = bacc.Bacc("TRN2", target_bir_lowering=False, num_devices=NC)
    f32, bf16 = mybir.dt.float32, mybir.dt.bfloat16

    lay_names = ["e0", "e1", "e2", "d0", "d1", "d2"]
    n_in_chunks = {"e0": 4, "e1": 8, "e2": 8, "d0": 2, "d1": 8, "d2": 8}

    di = {}  # dram inputs
    for n in lay_names:
        di[f"wih_{n}"] = nc.dram_tensor(
            f"wih_{n}", [128, n_in_chunks[n] * G], bf16, kind="ExternalInput"
        )
        di[f"whh_{n}"] = nc.dram_tensor(f"whh_{n}", [128, 8 * G], bf16, kind="ExternalInput")
        di[f"b_{n}"] = nc.dram_tensor(f"b_{n}", [1, G], bf16, kind="ExternalInput")
    for n in ["w_mu", "w_lv", "w_out"]:
        di[n] = nc.dram_tensor(n, [128, 8 * D], bf16, kind="ExternalInput")
    for n in ["b_mu", "b_lv", "b_out"]:
        di[n] = nc.dram_tensor(n, [1, D], bf16, kind="ExternalInput")
    di["xT"] = nc.dram_tensor("xT", [T, 128, 2 * B], bf16, kind="ExternalInput")
    di["x_f"] = nc.dram_tensor("x_f", [T, B, D], f32, kind="ExternalInput")
    di["eps_f"] = nc.dram_tensor("eps_f", [T, B, D], f32, kind="ExternalInput")
    di["ones"] = nc.dram_tensor("ones", [1, B], bf16, kind="ExternalInput")
    di["ident"] = nc.dram_tensor("ident", [128, 128], bf16, kind="ExternalInput")

    out_mu = nc.dram_tensor("out_mu", [T, B, D], f32, kind="ExternalOutput")
    out_lv = nc.dram_tensor("out_lv", [T, B, D], f32, kind="ExternalOutput")
    out_z = nc.dram_tensor("out_z", [T, B, D], f32, kind="ExternalOutput")
    out_lg = nc.dram_tensor("out_lg", [T, B, D], f32, kind="ExternalOutput")

    rg = [list(range(NC))]
    n_steps = T if steps is None else steps

    with tile.TileContext(nc) as tc:
        with (
            tc.tile_pool(name="wpool", bufs=1) as wp,
            tc.tile_pool(name="state", bufs=1) as st,
            tc.tile_pool(name="xio", bufs=4) as xio,
            tc.tile_pool(name="tmp", bufs=2) as tp,
            tc.tile_pool(name="psg", bufs=2, space="PSUM") as psg,
            tc.tile_pool(name="psh", bufs=2, space="PSUM") as psh,
            tc.tile_pool(name="pst", bufs=3, space="PSUM") as pst,
            tc.tile_pool(name="dio", bufs=2, space="DRAM") as dio,
        ):
            # ---- load weights into SBUF (persistent) ----
            w = {}
            for name, dt_ in list(di.items()):
                if name in ("xT", "x_f", "eps_f"):
                    continue
                shape = list(di[name].shape)
                t = wp.tile(shape, di[name].dtype, tag=f"w_{name}", name=f"w_{name}")
                nc.sync.dma_start(t[:], di[name][:])
                w[name] = t

            # ---- persistent state ----
            c_st = {}
            g_h = {}
            for n in lay_names:
                c_st[n] = st.tile([B, SL], f32, tag=f"c_{n}", name=f"c_{n}")
                nc.vector.memset(c_st[n][:], 0.0)
                g_h[n] = st.tile([128, NC * B], bf16, tag=f"gh_{n}", name=f"gh_{n}")
                nc.vector.memset(g_h[n][:], 0.0)
            xhatT = st.tile([128, 2 * B], bf16, tag="xhatT", name="xhatT")

            ident64 = w["ident"][0:64, 0:64]

            def transpose_to(dst_ap, src_ap):
                """src [64,128] sbuf -> dst [128,64] sbuf (via PE + copy)."""
                pt = pst.tile([128, B], bf16, tag="pt", name="pt")
                nc.tensor.transpose(pt[:], src_ap, ident64)
                nc.vector.tensor_copy(dst_ap, pt[:])

            # ---- prologue: xhatT for t=0  (x_hat = x_0 - sigmoid(0)) ----
            xf0 = xio.tile([B, D], f32, tag="xf", name="xf")
            nc.sync.dma_start(xf0[:], di["x_f"][0])
            xhat_bf0 = tp.tile([B, D], bf16, tag="xhat_bf", name="xhat_bf")
            nc.vector.tensor_scalar_add(xhat_bf0[:], xf0[:], -0.5)
            for cix in range(2):
                transpose_to(
                    xhatT[:, cix * B : (cix + 1) * B],
                    xhat_bf0[:, cix * 128 : (cix + 1) * 128],
                )

            def emit_gates(name, psum, in_stat):
                """Accumulate bias + recurrent + input terms into psum halves.

                in_stat: list of (ap_128x64,) stationary chunks for the input
                term, chunk c contracts with wih[:, c*G:(c+1)*G].
                """
                whh, wih, b = w[f"whh_{name}"], w[f"wih_{name}"], w[f"b_{name}"]
                # bias via K=1 matmul (clears psum, start=True)
                nc.tensor.matmul(
                    psum[:, :], w["ones"][0:1, 0:B], b[0:1, :], start=True, stop=False
                )
                ghl = g_h[name]
                n_in = len(in_stat)
                for cix in range(8):
                    nc.tensor.matmul(
                        psum[:, :],
                        ghl[:, 64 * cix : 64 * cix + 64],
                        whh[:, cix * G : (cix + 1) * G],
                        start=False,
                        stop=(cix == 7 and n_in == 0),
                    )
                for cix, stat in enumerate(in_stat):
                    nc.tensor.matmul(
                        psum[:, :],
                        stat,
                        wih[:, cix * G : (cix + 1) * G],
                        start=False,
                        stop=(cix == n_in - 1),
                    )

            def emit_cell(name, psum):
                """gates psum -> new h (bf16 [64,128]) ; updates c state."""
                if cell_mode == "stub":
                    h_bf = tp.tile([B, 128], bf16, tag="h_bf", name="h_bf")
                    nc.vector.tensor_copy(h_bf[:], psum[:, 0:128])
                    return h_bf
                sig_if = tp.tile([B, 256], f32, tag="sig_if", name="sig_if")
                nc.scalar.activation(sig_if[:], psum[:, 0:256], AF.Sigmoid)
                tanh_g = tp.tile([B, 128], f32, tag="tanh_g", name="tanh_g")
                nc.scalar.activation(tanh_g[:], psum[:, 256:384], AF.Tanh)
                sig_o = tp.tile([B, 128], f32, tag="sig_o", name="sig_o")
                nc.scalar.activation(sig_o[:], psum[:, 384:512], AF.Sigmoid)
                t1 = tp.tile([B, 128], f32, tag="t1", name="t1")
                nc.vector.tensor_mul(t1[:], sig_if[:, 0:128], tanh_g[:])
                t2 = tp.tile([B, 128], f32, tag="t2", name="t2")
                nc.vector.tensor_mul(t2[:], sig_if[:, 128:256], c_st[name][:])
                nc.vector.tensor_add(c_st[name][:], t1[:], t2[:])
                tanh_c = tp.tile([B, 128], f32, tag="tanh_c", name="tanh_c")
                nc.scalar.activation(tanh_c[:], c_st[name][:], AF.Tanh)
                h_bf = tp.tile([B, 128], bf16, tag="h_bf", name="h_bf")
                nc.vector.tensor_mul(h_bf[:], sig_o[:], tanh_c[:])
                return h_bf

            def emit_gather(name, h_bf):
                send = tp.tile([128, B], bf16, tag="send", name="send")
                transpose_to(send[:], h_bf[:])
                if gather_mode == "none":
                    # timing-only variant: pretend every core's slice is ours
                    for s in range(NC):
                        nc.vector.tensor_copy(g_h[name][:, s * B : (s + 1) * B], send[:])
                    return
                ib = dio.tile([128, B], bf16, tag="ib", name="ib")
                nc.sync.dma_start(ib[:], send[:])
                ob = dio.tile(
                    [NC, 128, B], bf16, tag="ob", name="ob",
                    addr_space="Shared" if gather_mode == "shared" else "Local",
                )
                nc.gpsimd.collective_compute(
                    "AllGather",
                    mybir.AluOpType.bypass,
                    replica_groups=rg,
                    ins=[ib.opt()],
                    outs=[ob.opt()],
                )
                if gather_mode == "cc1":
                    nc.sync.dma_start(
                        g_h[name][:].rearrange("p (s j) -> p s j", s=NC),
                        ob[:].rearrange("s p j -> p s j"),
                    )
                else:
                    for s in range(NC):
                        nc.sync.dma_start(g_h[name][:, s * B : (s + 1) * B], ob[s])

            def emit_head(wname, bname, stat_buf, psum):
                nc.tensor.matmul(
                    psum[:, :], w["ones"][0:1, 0:B], w[bname][0:1, :],
                    start=True, stop=False,
                )
                for cix in range(8):
                    nc.tensor.matmul(
                        psum[:, :],
                        stat_buf[:, 64 * cix : 64 * cix + 64],
                        w[wname][:, cix * D : (cix + 1) * D],
                        start=False,
                        stop=(cix == 7),
                    )

            # ================= time loop =================
            for t in range(n_steps):
                xT_t = xio.tile([128, 2 * B], bf16, tag="xT", name="xT")
                nc.sync.dma_start(xT_t[:], di["xT"][t])
                eps_t = xio.tile([B, D], f32, tag="eps", name="eps")
                nc.sync.dma_start(eps_t[:], di["eps_f"][t])

                # ---- encoder ----
                for li, name in enumerate(["e0", "e1", "e2"]):
                    if name == "e0":
                        in_stat = [
                            xT_t[:, 0:64], xT_t[:, 64:128],
                            xhatT[:, 0:64], xhatT[:, 64:128],
                        ]
                    else:
                        prev = g_h[["e0", "e1"][li - 1]]
                        in_stat = [prev[:, 64 * cx : 64 * cx + 64] for cx in range(8)]
                    psum = psg.tile([B, G], f32, tag="psg", name="psg")
                    emit_gates(name, psum, in_stat)
                    h_bf = emit_cell(name, psum)
                    emit_gather(name, h_bf)

                # ---- heads: mu, logvar, z ----
                pm = psh.tile([B, D], f32, tag="psh", name="psh")
                emit_head("w_mu", "b_mu", g_h["e2"], pm)
                mu_sb = tp.tile([B, D], f32, tag="mu_sb", name="mu_sb")
                nc.vector.tensor_copy(mu_sb[:], pm[:, :])
                nc.sync.dma_start(out_mu[t], mu_sb[:])

                pl = psh.tile([B, D], f32, tag="psh", name="psh")
                emit_head("w_lv", "b_lv", g_h["e2"], pl)
                lv_sb = tp.tile([B, D], f32, tag="lv_sb", name="lv_sb")
                nc.vector.tensor_copy(lv_sb[:], pl[:, :])
                nc.sync.dma_start(out_lv[t], lv_sb[:])

                std_t = tp.tile([B, D], f32, tag="std", name="std")
                nc.scalar.activation(std_t[:], lv_sb[:], AF.Exp, scale=0.5)
                tz = tp.tile([B, D], f32, tag="tz", name="tz")
                nc.vector.tensor_mul(tz[:], eps_t[:], std_t[:])
                z_sb = tp.tile([B, D], f32, tag="z_sb", name="z_sb")
                nc.vector.tensor_add(z_sb[:], mu_sb[:], tz[:])
                nc.sync.dma_start(out_z[t], z_sb[:])
                z_bf = tp.tile([B, D], bf16, tag="z_bf", name="z_bf")
                nc.vector.tensor_copy(z_bf[:], z_sb[:])
                zT = tp.tile([128, 2 * B], bf16, tag="zT", name="zT")
                for cix in range(2):
                    transpose_to(
                        zT[:, cix * B : (cix + 1) * B],
                        z_bf[:, cix * 128 : (cix + 1) * 128],
                    )

                # ---- decoder ----
                for li, name in enumerate(["d0", "d1", "d2"]):
                    if name == "d0":
                        in_stat = [zT[:, 0:64], zT[:, 64:128]]
                    else:
                        prev = g_h[["d0", "d1"][li - 1]]
                        in_stat = [prev[:, 64 * cx : 64 * cx + 64] for cx in range(8)]
                    psum = psg.tile([B, G], f32, tag="psg", name="psg")
                    emit_gates(name, psum, in_stat)
                    h_bf = emit_cell(name, psum)
                    emit_gather(name, h_bf)

                # ---- output head ----
                po = psh.tile([B, D], f32, tag="psh", name="psh")
                emit_head("w_out", "b_out", g_h["d2"], po)
                lg_sb = tp.tile([B, D], f32, tag="lg_sb", name="lg_sb")
                nc.vector.tensor_copy(lg_sb[:], po[:, :])
                nc.sync.dma_start(out_lg[t], lg_sb[:])

                # ---- x_hat for t+1 ----
                if t + 1 < n_steps:
                    sig_lg = tp.tile([B, D], f32, tag="sig_lg", name="sig_lg")
                    nc.scalar.activation(sig_lg[:], lg_sb[:], AF.Sigmoid)
                    xf_n = xio.tile([B, D], f32, tag="xf", name="xf")
                    nc.sync.dma_start(xf_n[:], di["x_f"][t + 1])
                    xhat_bf = tp.tile([B, D], bf16, tag="xhat_bf", name="xhat_bf")
                    nc.vector.tensor_sub(xhat_bf[:], xf_n[:], sig_lg[:])
                    for cix in range(2):
                        transpose_to(
                            xhatT[:, cix * B : (cix + 1) * B],
                            xhat_bf[:, cix * 128 : (cix + 1) * 128],
                        )

    nc.compile()
    return nc


_CACHE = {}


def run(inputs, T=T_FULL, trace=False):
    if T not in _CACHE:
        _CACHE[T] = build(T)
    nc = _CACHE[T]
    in_maps = [prepare_core_inputs(inputs, k, T) for k in range(NC)]
    res = run_bass_kernel_spmd(nc, in_maps, core_ids=list(range(NC)), trace=trace)
    r = res.results[0]
    sig = lambda v: 1.0 / (1.0 + np.exp(-v))
    tb = lambda a: np.ascontiguousarray(np.swapaxes(a, 0, 1))
    logits = tb(r["out_lg"])
    return (
        sig(logits).astype(np.float32),
        logits,
        tb(r["out_mu"]),
        tb(r["out_lv"]),
        tb(r["out_z"]),
    )


def kernel(**inputs):
    inputs = {k: np.asarray(v) for k, v in inputs.items()}
    return run(inputs, T=T_FULL)


if __name__ == "__main__":
    rng = np.random.default_rng(0)
    s = 0.05
    inputs = {
        "x": rng.standard_normal((B, T_FULL, D)).astype(np.float32),
        "eps": rng.standard_normal((B, T_FULL, D)).astype(np.float32),
        "enc_Wih0": (rng.standard_normal((4 * H, 2 * D)) * s).astype(np.float32),
        "enc_Wih": (rng.standard_normal((L - 1, 4 * H, H)) * s).astype(np.float32),
        "enc_Whh": (rng.standard_normal((L, 4 * H, H)) * s).astype(np.float32),
        "enc_b": (rng.standard_normal((L, 4 * H)) * s).astype(np.float32),
        "W_mu": (rng.standard_normal((D, H)) * s).astype(np.float32),
        "b_mu": np.zeros(D, np.float32),
        "W_logvar": (rng.standard_normal((D, H)) * s).astype(np.float32),
        "b_logvar": np.zeros(D, np.float32),
        "dec_Wih0": (rng.standard_normal((4 * H, D)) * s).astype(np.float32),
        "dec_Wih": (rng.standard_normal((L - 1, 4 * H, H)) * s).astype(np.float32),
        "dec_Whh": (rng.standard_normal((L, 4 * H, H)) * s).astype(np.float32),
        "dec_b": (rng.standard_normal((L, 4 * H)) * s).astype(np.float32),
        "W_out": (rng.standard_normal((D, H)) * s).astype(np.float32),
        "b_out": np.zeros(D, np.float32),
    }
    T_dbg = int(sys.argv[1]) if len(sys.argv) > 1 else 2
    outs = run(inputs, T=T_dbg)
    from numerics_check import run as np_run

    exp = np_run(inputs, lambda a: np.asarray(a, np.float32), T=T_dbg)
    for n, a in zip(["rec", "logits", "mu", "logvar", "z"], outs):
        r = exp[n][:, :T_dbg]
        a = a[:, :T_dbg]
        rel = np.linalg.norm(a - r) / max(np.linalg.norm(r), 1e-30)
        print(f"{n:8s} rel={rel:.3e} maxabs={np.abs(a - r).max():.3e}")

